# revision 1
# baseline (speedup 1.0000x reference)
"""Trainium2 Bass kernel for nn_LoRALayer: out = x @ W.T + b + 2.0*(x@A.T)@B.T.

Strategy: 8-way data-parallel over the token dim (N=8192 -> 1024/core).
Per core, a Tile-framework kernel computes the full [1024, 4096] output
shard with fp32r matmuls (full-rate fp32 on the PE at N>=256):

  - x and W are PE-transposed on chip into contraction-major (i-major)
    fp32r tiles (DMA transpose is 2-byte only, so fp32 uses the PE path).
  - The LoRA term and the bias are folded into the same PSUM accumulation
    as the main matmul: per output tile, one extra K=17 matmul with
    lhsT = [2*(x@A.T).T ; ones] and rhs = [B.T ; b].
"""

import os

import numpy as np

try:
    import concourse.bass as bass  # noqa: F401
except ImportError:  # pragma: no cover
    import sys

    sys.path.insert(0, "/opt/trn_rl_repo")
    import concourse.bass as bass  # noqa: F401

import concourse.tile as tile
from concourse import bacc, mybir
from concourse.bass_utils import run_bass_kernel_spmd
from concourse.masks import make_identity

P = 128
N_CORES = 8
N_TOK = 8192
NT = N_TOK // N_CORES  # tokens per core (1024)
KD = 4096  # in_features (contraction)
OD = 4096  # out_features
R = 16
SCALING = 2.0

KT = KD // P  # 32 k-tiles
MT = NT // P  # 8 token tiles per core
SLICES = [384] * 10 + [256]  # out-feature slice widths (psum-group free dim)
ICH = 1024  # natural-layout staging chunk (free dim)

F32 = mybir.dt.float32
F32R = mybir.dt.float32r

_NC_CACHE = None


def _build():
    from contextlib import ExitStack

    nc = bacc.Bacc("TRN2", target_bir_lowering=False, debug=False,
                   num_devices=N_CORES)
    x_d = nc.dram_tensor("x", [NT, KD], F32, kind="ExternalInput").ap()
    w_d = nc.dram_tensor("W", [OD, KD], F32, kind="ExternalInput").ap()
    b_d = nc.dram_tensor("b", [OD], F32, kind="ExternalInput").ap()
    a_d = nc.dram_tensor("lora_A", [R, KD], F32, kind="ExternalInput").ap()
    bb_d = nc.dram_tensor("lora_B", [OD, R], F32, kind="ExternalInput").ap()
    out_d = nc.dram_tensor("out", [NT, OD], F32, kind="ExternalOutput").ap()

    with tile.TileContext(nc) as tc, ExitStack() as ctx:
        const = ctx.enter_context(tc.tile_pool(name="const", bufs=1))
        nat = ctx.enter_context(tc.tile_pool(name="nat", bufs=4))
        xt_pool = ctx.enter_context(tc.tile_pool(name="xt", bufs=KT))
        wt_pool = ctx.enter_context(tc.tile_pool(name="wt", bufs=2 * KT + 4))
        at_pool = ctx.enter_context(tc.tile_pool(name="at", bufs=KT))
        t1_pool = ctx.enter_context(tc.tile_pool(name="t1", bufs=1))
        btb_pool = ctx.enter_context(tc.tile_pool(name="btb", bufs=3))
        osb_pool = ctx.enter_context(tc.tile_pool(name="osb", bufs=2))
        ps_tr = ctx.enter_context(tc.tile_pool(name="ps_tr", bufs=2, space="PSUM"))
        ps_c = ctx.enter_context(tc.tile_pool(name="ps_c", bufs=2, space="PSUM"))
        ps_out = ctx.enter_context(tc.tile_pool(name="ps_out", bufs=4, space="PSUM"))

        ident = const.tile([P, P], F32)
        make_identity(nc, ident[:])

        # b as [128p, 32a]: b[a*128 + p] at (p, a)
        b_all = const.tile([P, OD // P], F32, name="b_all")
        nc.sync.dma_start(b_all[:], b_d.rearrange("(a p) -> p a", p=P))

        # ---- Phase A: lora_A -> AT tiles [128i, 16r] (x SCALING), b rows ----
        at = []
        for ca in range(KD // ICH):
            ach = nat.tile([R, ICH], F32, tag="nat")
            nc.sync.dma_start(ach[:], a_d[:, ca * ICH:(ca + 1) * ICH])
            for j in range(ICH // P):
                pt = ps_tr.tile([P, R], F32, tag="pt")
                nc.tensor.transpose(pt[:], ach[:, j * P:(j + 1) * P],
                                    ident[0:R, 0:R])
                t = at_pool.tile([P, R], F32R, tag="at")
                nc.scalar.mul(t[:], pt[:], SCALING)
                at.append(t)

        # ---- Phase B: x -> xT tiles [128i, 1024t] fp32r (full cache) ----
        xt = [xt_pool.tile([P, NT], F32R, tag="xt", name=f"xt{_k}")
              for _k in range(KT)]
        for ic in range(KD // ICH):
            for mc in range(MT):
                xch = nat.tile([P, ICH], F32, tag="nat")
                nc.sync.dma_start(
                    xch[:], x_d[mc * P:(mc + 1) * P, ic * ICH:(ic + 1) * ICH])
                for j in range(ICH // P):
                    k = ic * (ICH // P) + j
                    pt = ps_tr.tile([P, P], F32, tag="pt")
                    nc.tensor.transpose(pt[:], xch[:, j * P:(j + 1) * P],
                                        ident[:])
                    nc.vector.tensor_copy(xt[k][:, mc * P:(mc + 1) * P], pt[:])

        # ---- Phase C: T1 = [2*(x@A.T).T ; ones] as [17, 1024] fp32r ----
        # t1.T computed directly: psum [16r, 512t] = AT.T @ xT (N=512 wide).
        # The ones row (partition 16) is written by a SBUF->SBUF DMA, since
        # compute engines cannot address a partition base of 16.
        t1 = t1_pool.tile([32, NT], F32R, tag="t1")
        for ts in range(NT // 512):
            pc = ps_c.tile([R, 512], F32, tag="pc")
            for k in range(KT):
                nc.tensor.matmul(pc[:], at[k][:],
                                 xt[k][:, ts * 512:(ts + 1) * 512],
                                 start=(k == 0), stop=(k == KT - 1))
            nc.vector.tensor_copy(t1[0:R, ts * 512:(ts + 1) * 512], pc[:])
        ones_f = nat.tile([1, NT], F32, tag="ones", bufs=1)
        nc.any.memset(ones_f[:], 1.0)
        ones_r = nat.tile([1, NT], F32R, tag="ones_r", bufs=1)
        nc.scalar.copy(ones_r[:], ones_f[:])
        nc.sync.dma_start(t1[R:R + 1, :], ones_r[:])

        # ---- Phase D: out.T orientation over o-tiles of 128 ----
        # Per o-tile: two [128o, 512t] PSUM groups (token halves). The
        # stationary operand wt[k] [128i, 128o] is shared by the two N=512
        # matmuls, so its weight load hides under the 213ns streams. W.T
        # tiles for o-tile ot+1 are transposed just-in-time, interleaved
        # 1:2 with the matmul stream (wt pool holds two o-tiles' worth).
        NOT = OD // P  # 32 o-tiles
        TSL = NT // 512  # 2 token halves

        wt_tiles = {}  # (ot, k) -> tile

        def _build_wt(ot, k):
            ic = k // (ICH // P)
            j = k % (ICH // P)
            wch = wstage.get((ot, ic))
            if wch is None:
                wch = nat.tile([P, ICH], F32, tag="nat",
                               name=f"wch{ot}_{ic}")
                nc.sync.dma_start(
                    wch[:],
                    w_d[ot * P:(ot + 1) * P, ic * ICH:(ic + 1) * ICH])
                wstage[(ot, ic)] = wch
            pt = ps_tr.tile([P, P], F32, tag="pt")
            nc.tensor.transpose(pt[:], wch[:, j * P:(j + 1) * P], ident[:])
            wt = wt_pool.tile([P, P], F32R, tag="wt", name=f"wt{ot}_{k}")
            nc.vector.tensor_copy(wt[:], pt[:])
            wt_tiles[(ot, k)] = wt

        def _build_btb(ot):
            bn = nat.tile([P, 32], F32, tag="t1n", bufs=2)
            nc.any.memset(bn[:], 0.0)
            nc.sync.dma_start(bn[:, 0:R], bb_d[ot * P:(ot + 1) * P, :])
            nc.vector.tensor_copy(bn[:, R:R + 1], b_all[:, ot:ot + 1])
            pt = ps_tr.tile([32, P], F32, tag="pt")
            nc.tensor.transpose(pt[:], bn[:], ident[:])
            btb = btb_pool.tile([32, P], F32R, tag="btb", name=f"btb{ot}")
            nc.vector.tensor_copy(btb[:], pt[:])
            return btb

        wstage = {}
        # prologue: o-tile 0's weights and btb
        btb_cur = _build_btb(0)
        for k in range(KT):
            _build_wt(0, k)

        for ot in range(NOT):
            btb_next = _build_btb(ot + 1) if ot + 1 < NOT else None
            pos = [ps_out.tile([P, 512], F32, tag="po", name=f"po{ot}_{t}")
                   for t in range(TSL)]
            for k in range(KT):
                wt = wt_tiles.pop((ot, k))
                for t in range(TSL):
                    nc.tensor.matmul(pos[t][:], wt[:],
                                     xt[k][:, t * 512:(t + 1) * 512],
                                     start=(k == 0), stop=False)
                if ot + 1 < NOT:
                    _build_wt(ot + 1, k)
            for t in range(TSL):
                nc.tensor.matmul(pos[t][:], btb_cur[0:R + 1, :],
                                 t1[0:R + 1, t * 512:(t + 1) * 512],
                                 start=False, stop=True)
            # evict: psum [128o, 512t] -> SBUF -> PE-transpose per 128t block
            for t in range(TSL):
                otb = osb_pool.tile([P, 512], F32, tag="otb", bufs=3)
                nc.scalar.copy(otb[:], pos[t][:])
                for j in range(512 // P):
                    pt = ps_tr.tile([P, P], F32, tag="pt")
                    nc.tensor.transpose(pt[:], otb[:, j * P:(j + 1) * P],
                                        ident[:])
                    osb = osb_pool.tile([P, P], F32, tag="osb", bufs=4)
                    nc.vector.tensor_copy(osb[:], pt[:])
                    nc.sync.dma_start(
                        out_d[t * 512 + j * P:t * 512 + (j + 1) * P,
                              ot * P:(ot + 1) * P], osb[:])
            btb_cur = btb_next

    nc.compile()
    return nc


def _get_nc():
    global _NC_CACHE
    if _NC_CACHE is None:
        _NC_CACHE = _build()
    return _NC_CACHE


def kernel(x, W, b, lora_A, lora_B):
    nc = _get_nc()
    x = np.ascontiguousarray(x, dtype=np.float32)
    W = np.ascontiguousarray(W, dtype=np.float32)
    b = np.ascontiguousarray(b, dtype=np.float32)
    lora_A = np.ascontiguousarray(lora_A, dtype=np.float32)
    lora_B = np.ascontiguousarray(lora_B, dtype=np.float32)
    in_maps = [
        {
            "x": x[c * NT:(c + 1) * NT],
            "W": W,
            "b": b,
            "lora_A": lora_A,
            "lora_B": lora_B,
        }
        for c in range(N_CORES)
    ]
    res = run_bass_kernel_spmd(nc, in_maps, core_ids=list(range(N_CORES)),
                               trace=bool(int(os.environ.get("LORA_TRACE", "0"))))
    kernel.last_results = res
    return np.concatenate([res.results[c]["out"] for c in range(N_CORES)], axis=0)


if __name__ == "__main__":
    rng = np.random.default_rng(0)
    x = rng.standard_normal((N_TOK, KD), dtype=np.float32)
    W = (rng.standard_normal((OD, KD)) * 0.02).astype(np.float32)
    b = (rng.standard_normal(OD) * 0.02).astype(np.float32)
    A = (rng.standard_normal((R, KD)) * 0.02).astype(np.float32)
    B = (rng.standard_normal((OD, R)) * 0.02).astype(np.float32)
    out = kernel(x=x, W=W, b=b, lora_A=A, lora_B=B)
    ref = x.astype(np.float64) @ W.T.astype(np.float64) + b + SCALING * (
        (x.astype(np.float64) @ A.T.astype(np.float64)) @ B.T.astype(np.float64))
    rel = np.linalg.norm(out - ref) / np.linalg.norm(ref)
    print("rel_l2:", rel)



# revision 3
# speedup vs baseline: 1.5258x; 1.5258x over previous
"""Trainium2 Bass kernel for nn_LoRALayer: out = x @ W.T + b + 2.0*(x@A.T)@B.T.

Strategy: 8-way data-parallel over the token dim (N=8192 -> 1024/core),
all-bf16 datapath (inputs host-cast to bf16; fp32 PSUM accumulation;
fp32 output):

  - x, W and lora_A are transposed into contraction-major (i-major)
    layout by the DMA XBAR (dma_start_transpose, 2-byte dtype), so the
    PE spends its cycles only on matmul streams.
  - psum orientation is [token, out_f]: evicted tiles DMA straight to
    the output with no transpose.
  - The bias and the LoRA term fold into each PSUM accumulation group
    as one extra K=17 matmul: lhsT = [2*(x@A.T).T ; ones],
    rhs = [B.T ; b].
"""

import os

import numpy as np

try:
    import concourse.bass as bass  # noqa: F401
except ImportError:  # pragma: no cover
    import sys

    sys.path.insert(0, "/opt/trn_rl_repo")
    import concourse.bass as bass  # noqa: F401

import ml_dtypes
import concourse.tile as tile
from concourse import bacc, mybir
from concourse.bass_utils import run_bass_kernel_spmd
from concourse.masks import make_identity

P = 128
N_CORES = 8
N_TOK = 8192
NT = N_TOK // N_CORES  # tokens per core (1024)
KD = 4096  # in_features (contraction)
OD = 4096  # out_features
R = 16
SCALING = 2.0

KT = KD // P  # 32 k-tiles
MT = NT // P  # 8 token tiles per core
NOP = 8  # out-feature panels
OPW = OD // NOP  # 512
KC = 4  # k-tiles per transpose-DMA chunk
NKC = KT // KC  # 8 chunks

F32 = mybir.dt.float32
BF16 = mybir.dt.bfloat16

_NC_CACHE = None


def _build():
    from contextlib import ExitStack

    nc = bacc.Bacc("TRN2", target_bir_lowering=False, debug=False,
                   num_devices=N_CORES)
    x_d = nc.dram_tensor("x", [NT, KD], BF16, kind="ExternalInput").ap()
    w_d = nc.dram_tensor("W", [OD, KD], BF16, kind="ExternalInput").ap()
    b_d = nc.dram_tensor("b", [OD], BF16, kind="ExternalInput").ap()
    a_d = nc.dram_tensor("lora_A", [R, KD], BF16, kind="ExternalInput").ap()
    bb_d = nc.dram_tensor("lora_B", [OD, R], BF16, kind="ExternalInput").ap()
    out_d = nc.dram_tensor("out", [NT, OD], F32, kind="ExternalOutput").ap()

    with tile.TileContext(nc) as tc, ExitStack() as ctx:
        const = ctx.enter_context(tc.tile_pool(name="const", bufs=1))
        xt_pool = ctx.enter_context(tc.tile_pool(name="xt", bufs=1))
        wp_pool = ctx.enter_context(tc.tile_pool(name="wp", bufs=2))
        small = ctx.enter_context(tc.tile_pool(name="small", bufs=1))
        osb_pool = ctx.enter_context(tc.tile_pool(name="osb", bufs=4))
        ps = ctx.enter_context(tc.tile_pool(name="ps", bufs=1, space="PSUM"))

        # ---- tiny loads first: B blocks, b row, identity ----
        ident = const.tile([P, P], BF16)
        make_identity(nc, ident[:])
        bsb = const.tile([P, KT, R], BF16, name="bsb")  # B (kb,p)-blocked
        nc.sync.dma_start(bsb[:], bb_d.rearrange("(kb p) r -> p kb r", p=P))
        btbT = const.tile([32, OD], BF16, name="btbT")  # [B.T ; b]
        nc.sync.dma_start(btbT[R:R + 1, :],
                          b_d.rearrange("(one o) -> one o", one=1))
        onesb = small.tile([1, NT], BF16, tag="ones")
        nc.any.memset(onesb[:], 1.0)
        t1sb = const.tile([32, NT], BF16, name="t1sb")  # [2*(x@A.T).T ; 1]
        nc.sync.dma_start(t1sb[R:R + 1, :], onesb[:])

        # ---- x -> xT [128i, k, t] via DMA XBAR ----
        xT = xt_pool.tile([P, KT, NT], BF16, name="xT")
        for c in range(NKC):
            nc.sync.dma_start_transpose(
                xT[:, c * KC:(c + 1) * KC, :],
                x_d[:, c * KC * P:(c + 1) * KC * P])

        # ---- A -> aT [128i, k, r], scaled by 2 ----
        aT = small.tile([P, KT, R], BF16, tag="aT")
        nc.sync.dma_start_transpose(aT[:], a_d)
        aTs = small.tile([P, KT, R], BF16, tag="aTs")
        nc.scalar.mul(aTs[:], aT[:], SCALING)

        # ---- W panel 0 -> wT [128i, k, o] via DMA XBAR ----
        wp_tiles = {}

        def issue_wp_chunk(op, c):
            wp = wp_tiles.get(op)
            if wp is None:
                wp = wp_pool.tile([P, KT, OPW], BF16, tag="wp",
                                  name=f"wp{op}")
                wp_tiles[op] = wp
            nc.sync.dma_start_transpose(
                wp[:, c * KC:(c + 1) * KC, :],
                w_d[op * OPW:(op + 1) * OPW, c * KC * P:(c + 1) * KC * P])

        for c in range(NKC):
            issue_wp_chunk(0, c)

        # ---- PE prologue: btbT rows 0..15 via PE transposes of B ----
        for kb in range(KT):
            pt = ps.tile([R, P], BF16, tag="bt", bufs=1)
            nc.tensor.transpose(pt[:], bsb[:, kb, :], ident[:])
            nc.vector.tensor_copy(btbT[0:R, kb * P:(kb + 1) * P], pt[:])

        # ---- t1 rows 0..15: psum [16, 512] = (2A).T-major @ xT ----
        for h in range(2):
            pc = ps.tile([R, 512], F32, tag="t1", bufs=1)
            for k in range(KT):
                nc.tensor.matmul(pc[:], aTs[:, k, :],
                                 xT[:, k, h * 512:(h + 1) * 512],
                                 start=(k == 0), stop=(k == KT - 1))
            nc.scalar.copy(t1sb[0:R, h * 512:(h + 1) * 512], pc[:])

        # ---- main: per (o-panel, token-tile) psum group of 33 matmuls ----
        for op in range(NOP):
            wp = wp_tiles.pop(op)
            for tt in range(MT):
                po = ps.tile([P, OPW], F32, tag="po", bufs=6)
                for k in range(KT):
                    nc.tensor.matmul(po[:], xT[:, k, tt * P:(tt + 1) * P],
                                     wp[:, k, :],
                                     start=(k == 0), stop=False)
                nc.tensor.matmul(po[:], t1sb[0:R + 1, tt * P:(tt + 1) * P],
                                 btbT[0:R + 1, op * OPW:(op + 1) * OPW],
                                 start=False, stop=True)
                if op + 1 < NOP:
                    issue_wp_chunk(op + 1, tt)
                osb = osb_pool.tile([P, OPW], F32, tag="osb")
                nc.scalar.copy(osb[:], po[:])
                nc.sync.dma_start(
                    out_d[tt * P:(tt + 1) * P, op * OPW:(op + 1) * OPW],
                    osb[:])

    nc.compile()
    return nc


def _get_nc():
    global _NC_CACHE
    if _NC_CACHE is None:
        _NC_CACHE = _build()
    return _NC_CACHE


def kernel(x, W, b, lora_A, lora_B):
    nc = _get_nc()
    bf = ml_dtypes.bfloat16
    x = np.ascontiguousarray(np.asarray(x, dtype=np.float32).astype(bf))
    W = np.ascontiguousarray(np.asarray(W, dtype=np.float32).astype(bf))
    b = np.ascontiguousarray(np.asarray(b, dtype=np.float32).astype(bf))
    lora_A = np.ascontiguousarray(
        np.asarray(lora_A, dtype=np.float32).astype(bf))
    lora_B = np.ascontiguousarray(
        np.asarray(lora_B, dtype=np.float32).astype(bf))
    in_maps = [
        {
            "x": x[c * NT:(c + 1) * NT],
            "W": W,
            "b": b,
            "lora_A": lora_A,
            "lora_B": lora_B,
        }
        for c in range(N_CORES)
    ]
    res = run_bass_kernel_spmd(nc, in_maps, core_ids=list(range(N_CORES)),
                               trace=bool(int(os.environ.get("LORA_TRACE", "0"))))
    kernel.last_results = res
    return np.concatenate([res.results[c]["out"] for c in range(N_CORES)],
                          axis=0)


if __name__ == "__main__":
    rng = np.random.default_rng(0)
    x = rng.standard_normal((N_TOK, KD), dtype=np.float32)
    W = (rng.standard_normal((OD, KD)) * 0.02).astype(np.float32)
    b = (rng.standard_normal(OD) * 0.02).astype(np.float32)
    A = (rng.standard_normal((R, KD)) * 0.02).astype(np.float32)
    B = (rng.standard_normal((OD, R)) * 0.02).astype(np.float32)
    out = kernel(x=x, W=W, b=b, lora_A=A, lora_B=B)
    ref = x.astype(np.float64) @ W.T.astype(np.float64) + b + SCALING * (
        (x.astype(np.float64) @ A.T.astype(np.float64)) @ B.T.astype(np.float64))
    rel = np.linalg.norm(out - ref) / np.linalg.norm(ref)
    print("rel_l2:", rel)
